# revision 38
# baseline (speedup 1.0000x reference)
"""AdaptiveCenterLoss on 8 TRN2 NeuronCores.

loss = mean_i ||features[i] - centers[labels[i]]||^2
     with B=131072, D=256, C=1000.

Strategy (data-parallel, memory-bound):
  - host-side, sort rows by label and pack them into 16-row blocks, each
    block sharing one label; partial blocks are padded with rows equal to
    that class's center (contributing exactly 0 to the sum)
  - shard the padded blocks across 8 cores x 128 partitions as J blocks
    per partition; per tile (one block per partition), ONE [128,1]-index
    indirect DMA gathers the 128 needed center rows (the HW DGE consumes
    one index per partition per call and its per-descriptor software cost
    makes per-row gathers ~16x more expensive, so block-level gathers
    after the sort are the key trick)
  - per tile: DVE subtract (center broadcast across the 16 slots via a
    stride-0 AP), ACT square + row-sum accumulate; the pipeline is paced
    by the feature DMA at ~350 GB/s per core, i.e. at the HBM roofline
  - each core outputs per-tile partial sums; host sums and divides by B
"""

import numpy as np

import concourse.bacc as bacc
import concourse.bass as bass
import concourse.mybir as mybir
import concourse.tile as tile
from concourse.bass_utils import run_bass_kernel_spmd

B, D, C = 131072, 256, 1000
N_CORES = 8
P = 128   # SBUF partitions
S = 16    # rows per block (one label per block)

_nc_cache = {}


def _build(J):
    """Per-core graph for J tiles (one S-row block per partition per tile)."""
    if J in _nc_cache:
        return _nc_cache[J]
    splits = [1] * J
    acc_cols = sum(splits)

    nc = bacc.Bacc()
    feats = nc.declare_dram_parameter(
        "features", [J * P * S, D], mybir.dt.float32, isOutput=False
    )
    labels = nc.declare_dram_parameter("labels", [P, J], mybir.dt.int32, isOutput=False)
    centers = nc.declare_dram_parameter(
        "centers", [C, D], mybir.dt.float32, isOutput=False
    )
    out = nc.declare_dram_parameter(
        "out", [P, acc_cols], mybir.dt.float32, isOutput=True
    )

    # tile j, partition p, slot s <- feature row (j*128 + p)*S + s
    fview = feats[:].rearrange("(j p s) d -> p j s d", p=P, s=S)

    with tile.TileContext(nc) as tc:
        with (
            tc.tile_pool(name="lab", bufs=1) as lab_pool,
            tc.tile_pool(name="f", bufs=4) as f_pool,
            tc.tile_pool(name="c", bufs=4) as c_pool,
            tc.tile_pool(name="acc", bufs=1) as acc_pool,
        ):
            lab = lab_pool.tile([P, J], mybir.dt.int32)
            nc.sync.dma_start(out=lab[:], in_=labels[:])
            acc = acc_pool.tile([P, acc_cols], mybir.dt.float32)
            col = 0
            for t in range(J):
                H = splits[t]
                SH = S // H
                f_t = f_pool.tile([P, S * D], mybir.dt.float32, tag="f")
                dma_eng = nc.sync if t % 2 == 0 else nc.scalar
                for h in range(H):
                    dma_eng.dma_start(
                        out=f_t[:, h * SH * D : (h + 1) * SH * D].rearrange(
                            "p (s d) -> p s d", s=SH
                        ),
                        in_=fview[:, t, h * SH : (h + 1) * SH, :],
                    )
                c_s = c_pool.tile([P, D], mybir.dt.float32, tag="c")
                nc.gpsimd.indirect_dma_start(
                    out=c_s[:],
                    out_offset=None,
                    in_=centers[:],
                    in_offset=bass.IndirectOffsetOnAxis(ap=lab[:, t : t + 1], axis=0),
                )
                c_b = (
                    c_s[:]
                    .rearrange("p (s d) -> p s d", s=1)
                    .to_broadcast([P, SH, D])
                )
                for h in range(H):
                    fh = f_t[:, h * SH * D : (h + 1) * SH * D]
                    nc.vector.tensor_tensor(
                        out=fh.rearrange("p (s d) -> p s d", s=SH),
                        in0=fh.rearrange("p (s d) -> p s d", s=SH),
                        in1=c_b,
                        op=mybir.AluOpType.subtract,
                    )
                    nc.scalar.activation(
                        out=fh,
                        in_=fh,
                        func=mybir.ActivationFunctionType.Square,
                        accum_out=acc[:, col : col + 1],
                    )
                    col += 1
            nc.sync.dma_start(out=out[:], in_=acc[:])
    nc.finalize()
    _nc_cache[J] = nc
    return nc


def _prepare(features, centers, labels):
    """Sort rows by label into padded S-row blocks; returns per-core maps + J."""
    features = np.ascontiguousarray(np.asarray(features), dtype=np.float32)
    centers = np.ascontiguousarray(np.asarray(centers), dtype=np.float32)
    labels = np.asarray(labels).astype(np.int32)

    counts = np.bincount(labels, minlength=C)          # [C]
    nblocks = -(-counts // S)                          # ceil(n_c / S) per class
    nb = int(nblocks.sum())
    group = N_CORES * P                                # blocks per slot across chip
    nb_pad = -(-nb // group) * group
    J = nb_pad // group                                # block-slots per partition

    # block labels, in sorted-class order; pad blocks use class 0
    block_labels = np.zeros(nb_pad, dtype=np.int32)
    block_labels[:nb] = np.repeat(np.arange(C, dtype=np.int32), nblocks)

    # every padded slot starts as its block's center row -> contributes 0
    fpad = centers[block_labels].repeat(S, axis=0).reshape(nb_pad * S, D)

    # scatter the real rows into their slots
    order = np.argsort(labels)
    labels_sorted = labels[order]
    class_row_start = np.concatenate(([0], np.cumsum(counts)[:-1]))
    class_slot_start = S * np.concatenate(([0], np.cumsum(nblocks)[:-1]))
    rank = np.arange(B) - class_row_start[labels_sorted]
    dst = class_slot_start[labels_sorted] + rank
    fpad[dst] = features[order]

    rows_core = J * P * S
    maps = []
    for k in range(N_CORES):
        fs = fpad[k * rows_core : (k + 1) * rows_core]
        # labW[p, j] = block_labels[(k*J + j)*128 + p]
        lw = np.ascontiguousarray(
            block_labels[k * J * P : (k + 1) * J * P].reshape(J, P).T
        )
        maps.append({"features": fs, "labels": lw, "centers": centers})
    return maps, J


def run(features, centers, labels, trace=False):
    """Run on 8 cores; returns (loss_scalar, BassKernelResults)."""
    maps, J = _prepare(features, centers, labels)
    nc = _build(J)
    res = run_bass_kernel_spmd(
        nc, maps, core_ids=list(range(N_CORES)), trace=trace
    )
    total = 0.0
    for r in res.results:
        total += float(np.asarray(r["out"]).astype(np.float64).sum())
    return np.float32(total / B), res


def kernel(features, centers, labels):
    last_err = None
    for _ in range(3):  # retry transient device errors
        try:
            loss, _ = run(features, centers, labels)
            return loss
        except Exception as e:  # noqa: BLE001
            last_err = e
    raise last_err


# revision 39
# speedup vs baseline: 1.0366x; 1.0366x over previous
"""AdaptiveCenterLoss on 8 TRN2 NeuronCores.

loss = mean_i ||features[i] - centers[labels[i]]||^2
     with B=131072, D=256, C=1000.

Strategy (data-parallel, memory-bound):
  - host-side, sort rows by label and pack them into 16-row blocks, each
    block sharing one label; partial blocks are padded with rows equal to
    that class's center (contributing exactly 0 to the sum)
  - shard the padded blocks across 8 cores x 128 partitions as J blocks
    per partition; per tile (one block per partition), ONE [128,1]-index
    indirect DMA gathers the 128 needed center rows (the HW DGE consumes
    one index per partition per call and its per-descriptor software cost
    makes per-row gathers ~16x more expensive, so block-level gathers
    after the sort are the key trick)
  - per tile: DVE subtract (center broadcast across the 16 slots via a
    stride-0 AP), ACT square + row-sum accumulate; the pipeline is paced
    by the feature DMA at ~350 GB/s per core, i.e. at the HBM roofline
  - each core outputs per-tile partial sums; host sums and divides by B
"""

import numpy as np

import concourse.bacc as bacc
import concourse.bass as bass
import concourse.mybir as mybir
import concourse.tile as tile
from concourse.bass_utils import run_bass_kernel_spmd

B, D, C = 131072, 256, 1000
N_CORES = 8
P = 128   # SBUF partitions
S = 16    # rows per block (one label per block)

_nc_cache = {}


def _build(J):
    """Per-core graph for J tiles (one S-row block per partition per tile)."""
    if J in _nc_cache:
        return _nc_cache[J]
    splits = [1] * J
    acc_cols = sum(splits)

    nc = bacc.Bacc()
    feats = nc.declare_dram_parameter(
        "features", [J * P * S, D], mybir.dt.float32, isOutput=False
    )
    labels = nc.declare_dram_parameter("labels", [P, J], mybir.dt.int32, isOutput=False)
    centers = nc.declare_dram_parameter(
        "centers", [C, D], mybir.dt.float32, isOutput=False
    )
    out = nc.declare_dram_parameter(
        "out", [P, acc_cols], mybir.dt.float32, isOutput=True
    )

    # tile j, partition p, slot s <- feature row (j*128 + p)*S + s
    fview = feats[:].rearrange("(j p s) d -> p j s d", p=P, s=S)

    with tile.TileContext(nc) as tc:
        with (
            tc.tile_pool(name="lab", bufs=1) as lab_pool,
            tc.tile_pool(name="f", bufs=4) as f_pool,
            tc.tile_pool(name="c", bufs=4) as c_pool,
            tc.tile_pool(name="acc", bufs=1) as acc_pool,
        ):
            lab = lab_pool.tile([P, J], mybir.dt.int32)
            nc.sync.dma_start(out=lab[:], in_=labels[:])
            acc = acc_pool.tile([P, acc_cols], mybir.dt.float32)
            col = 0
            for t in range(J):
                H = splits[t]
                SH = S // H
                f_t = f_pool.tile([P, S * D], mybir.dt.float32, tag="f")
                for h in range(H):
                    nc.sync.dma_start(
                        out=f_t[:, h * SH * D : (h + 1) * SH * D].rearrange(
                            "p (s d) -> p s d", s=SH
                        ),
                        in_=fview[:, t, h * SH : (h + 1) * SH, :],
                    )
                c_s = c_pool.tile([P, D], mybir.dt.float32, tag="c")
                nc.gpsimd.indirect_dma_start(
                    out=c_s[:],
                    out_offset=None,
                    in_=centers[:],
                    in_offset=bass.IndirectOffsetOnAxis(ap=lab[:, t : t + 1], axis=0),
                )
                c_b = (
                    c_s[:]
                    .rearrange("p (s d) -> p s d", s=1)
                    .to_broadcast([P, SH, D])
                )
                for h in range(H):
                    fh = f_t[:, h * SH * D : (h + 1) * SH * D]
                    nc.vector.tensor_tensor(
                        out=fh.rearrange("p (s d) -> p s d", s=SH),
                        in0=fh.rearrange("p (s d) -> p s d", s=SH),
                        in1=c_b,
                        op=mybir.AluOpType.subtract,
                    )
                    nc.scalar.activation(
                        out=fh,
                        in_=fh,
                        func=mybir.ActivationFunctionType.Square,
                        accum_out=acc[:, col : col + 1],
                    )
                    col += 1
            nc.sync.dma_start(out=out[:], in_=acc[:])
    nc.finalize()
    _nc_cache[J] = nc
    return nc


def _prepare(features, centers, labels):
    """Sort rows by label into padded S-row blocks; returns per-core maps + J."""
    features = np.ascontiguousarray(np.asarray(features), dtype=np.float32)
    centers = np.ascontiguousarray(np.asarray(centers), dtype=np.float32)
    labels = np.asarray(labels).astype(np.int32)

    counts = np.bincount(labels, minlength=C)          # [C]
    nblocks = -(-counts // S)                          # ceil(n_c / S) per class
    nb = int(nblocks.sum())
    group = N_CORES * P                                # blocks per slot across chip
    nb_pad = -(-nb // group) * group
    J = nb_pad // group                                # block-slots per partition

    # block labels, in sorted-class order; pad blocks use class 0
    block_labels = np.zeros(nb_pad, dtype=np.int32)
    block_labels[:nb] = np.repeat(np.arange(C, dtype=np.int32), nblocks)

    # every padded slot starts as its block's center row -> contributes 0
    fpad = centers[block_labels].repeat(S, axis=0).reshape(nb_pad * S, D)

    # scatter the real rows into their slots
    order = np.argsort(labels)
    labels_sorted = labels[order]
    class_row_start = np.concatenate(([0], np.cumsum(counts)[:-1]))
    class_slot_start = S * np.concatenate(([0], np.cumsum(nblocks)[:-1]))
    rank = np.arange(B) - class_row_start[labels_sorted]
    dst = class_slot_start[labels_sorted] + rank
    fpad[dst] = features[order]

    rows_core = J * P * S
    maps = []
    for k in range(N_CORES):
        fs = fpad[k * rows_core : (k + 1) * rows_core]
        # labW[p, j] = block_labels[(k*J + j)*128 + p]
        lw = np.ascontiguousarray(
            block_labels[k * J * P : (k + 1) * J * P].reshape(J, P).T
        )
        maps.append({"features": fs, "labels": lw, "centers": centers})
    return maps, J


def run(features, centers, labels, trace=False):
    """Run on 8 cores; returns (loss_scalar, BassKernelResults)."""
    maps, J = _prepare(features, centers, labels)
    nc = _build(J)
    res = run_bass_kernel_spmd(
        nc, maps, core_ids=list(range(N_CORES)), trace=trace
    )
    total = 0.0
    for r in res.results:
        total += float(np.asarray(r["out"]).astype(np.float64).sum())
    return np.float32(total / B), res


def kernel(features, centers, labels):
    last_err = None
    for _ in range(3):  # retry transient device errors
        try:
            loss, _ = run(features, centers, labels)
            return loss
        except Exception as e:  # noqa: BLE001
            last_err = e
    raise last_err


# revision 40
# speedup vs baseline: 1.1188x; 1.0793x over previous
"""AdaptiveCenterLoss on 8 TRN2 NeuronCores.

loss = mean_i ||features[i] - centers[labels[i]]||^2
     with B=131072, D=256, C=1000.

Strategy (data-parallel, memory-bound):
  - host-side, sort rows by label and pack them into one-label blocks;
    partial blocks are padded with rows equal to that class's center
    (contributing exactly 0 to the sum).  Each class's bulk goes into
    16-row blocks; a remainder of <= 8 rows goes into an 8-row block in
    trailing 8-slot tiles (halves the padding vs all-16 blocks).
  - shard the blocks across 8 cores x 128 partitions, one block per
    partition per tile; ONE [128,1]-index indirect DMA per tile gathers
    the 128 needed center rows (the HW DGE consumes one index per
    partition per call and costs ~10ns/descriptor of Q7 software time,
    so per-row gathers would cost ~164us/core -- the sort is the trick)
  - per tile: DVE subtract (center broadcast across the slots via a
    stride-0 AP), ACT square + fused row-sum accumulate; the pipeline is
    paced by the feature DMA at ~350 GB/s/core, i.e. the HBM roofline,
    and the small trailing tile drains it quickly
  - each core outputs per-tile partial sums; host sums and divides by B
"""

import numpy as np

import concourse.bacc as bacc
import concourse.bass as bass
import concourse.mybir as mybir
import concourse.tile as tile
from concourse.bass_utils import run_bass_kernel_spmd

B, D, C = 131072, 256, 1000
N_CORES = 8
P = 128

_nc_cache = {}


def _build(slots_list):
    """Per-core graph; tile t holds one slots_list[t]-row block per partition."""
    key = tuple(slots_list)
    if key in _nc_cache:
        return _nc_cache[key]
    T = len(slots_list)
    rows_core = P * sum(slots_list)

    nc = bacc.Bacc()
    feats = nc.declare_dram_parameter(
        "features", [rows_core, D], mybir.dt.float32, isOutput=False
    )
    labels = nc.declare_dram_parameter("labels", [P, T], mybir.dt.int32, isOutput=False)
    centers = nc.declare_dram_parameter(
        "centers", [C, D], mybir.dt.float32, isOutput=False
    )
    out = nc.declare_dram_parameter("out", [P, T], mybir.dt.float32, isOutput=True)

    fall = feats[:]

    with tile.TileContext(nc) as tc:
        with (
            tc.tile_pool(name="lab", bufs=1) as lab_pool,
            tc.tile_pool(name="f", bufs=4) as f_pool,
            tc.tile_pool(name="c", bufs=4) as c_pool,
            tc.tile_pool(name="acc", bufs=1) as acc_pool,
        ):
            lab = lab_pool.tile([P, T], mybir.dt.int32)
            nc.sync.dma_start(out=lab[:], in_=labels[:])
            acc = acc_pool.tile([P, T], mybir.dt.float32)
            rowbase = 0
            for t, slots in enumerate(slots_list):
                f_t = f_pool.tile([P, slots * D], mybir.dt.float32, tag="f")
                nc.sync.dma_start(
                    out=f_t[:].rearrange("p (s d) -> p s d", s=slots),
                    in_=fall[rowbase : rowbase + P * slots, :].rearrange(
                        "(p s) d -> p s d", p=P
                    ),
                )
                c_s = c_pool.tile([P, D], mybir.dt.float32, tag="c")
                nc.gpsimd.indirect_dma_start(
                    out=c_s[:],
                    out_offset=None,
                    in_=centers[:],
                    in_offset=bass.IndirectOffsetOnAxis(ap=lab[:, t : t + 1], axis=0),
                )
                c_b = (
                    c_s[:]
                    .rearrange("p (s d) -> p s d", s=1)
                    .to_broadcast([P, slots, D])
                )
                nc.vector.tensor_tensor(
                    out=f_t[:].rearrange("p (s d) -> p s d", s=slots),
                    in0=f_t[:].rearrange("p (s d) -> p s d", s=slots),
                    in1=c_b,
                    op=mybir.AluOpType.subtract,
                )
                nc.scalar.activation(
                    out=f_t[:],
                    in_=f_t[:],
                    func=mybir.ActivationFunctionType.Square,
                    accum_out=acc[:, t : t + 1],
                )
                rowbase += P * slots
            nc.sync.dma_start(out=out[:], in_=acc[:])
    nc.finalize()
    _nc_cache[key] = nc
    return nc


def _prepare(features, centers, labels):
    features = np.ascontiguousarray(np.asarray(features), dtype=np.float32)
    centers = np.ascontiguousarray(np.asarray(centers), dtype=np.float32)
    labels = np.asarray(labels).astype(np.int32)

    counts = np.bincount(labels, minlength=C)
    full = counts // 16
    rem = counts % 16
    # bulk 16-row blocks; remainders >8 get their own 16-block, <=8 an 8-block
    b16 = full + (rem > 8)
    b8 = ((rem > 0) & (rem <= 8)).astype(np.int64)
    N16, N8 = int(b16.sum()), int(b8.sum())
    group = N_CORES * P
    J16 = max(1, -(-N16 // group))
    J8 = max(1, -(-N8 // group)) if N8 else 0
    slots_list = [16] * J16 + [8] * J8
    rows_core = P * sum(slots_list)

    # block labels per region, class-major; pad blocks use class 0
    lab16 = np.zeros(J16 * group, dtype=np.int32)
    lab16[:N16] = np.repeat(np.arange(C, dtype=np.int32), b16)
    lab8 = np.zeros(J8 * group, dtype=np.int32)
    if N8:
        lab8[:N8] = np.repeat(np.arange(C, dtype=np.int32), b8)

    # global row start of each block position (order: core, tile, partition)
    def region_rows(nblk_core, blk_rows, base_off):
        # block j of core k starts at k*rows_core + base_off + j*blk_rows
        k = np.arange(N_CORES, dtype=np.int64)
        j = np.arange(nblk_core, dtype=np.int64)
        return (
            (k[:, None] * rows_core + base_off + j[None, :] * blk_rows)
            .reshape(-1)
        )

    rs16 = region_rows(J16 * P, 16, 0)
    rs8 = region_rows(J8 * P, 8, J16 * P * 16) if J8 else np.empty(0, np.int64)

    # init every slot with its block's center -> pad rows contribute 0
    fpad = np.empty((N_CORES * rows_core, D), dtype=np.float32)
    if J16:
        rows = (rs16[:, None] + np.arange(16)).ravel()
        fpad[rows] = centers[lab16].repeat(16, axis=0)
    if J8:
        rows = (rs8[:, None] + np.arange(8)).ravel()
        fpad[rows] = centers[lab8].repeat(8, axis=0)

    # scatter real rows
    order = np.argsort(labels)
    labels_sorted = labels[order]
    class_row_start = np.concatenate(([0], np.cumsum(counts)[:-1]))
    start16 = np.concatenate(([0], np.cumsum(b16)[:-1]))
    start8 = np.concatenate(([0], np.cumsum(b8)[:-1]))
    rank = np.arange(B) - class_row_start[labels_sorted]
    cap16 = 16 * b16[labels_sorted]
    in16 = rank < cap16
    dst = np.empty(B, dtype=np.int64)
    blk = start16[labels_sorted[in16]] + rank[in16] // 16
    dst[in16] = rs16[blk] + rank[in16] % 16
    n8m = ~in16
    if n8m.any():
        r8 = rank[n8m] - cap16[n8m]
        dst[n8m] = rs8[start8[labels_sorted[n8m]]] + r8
    fpad[dst] = features[order]

    maps = []
    T = len(slots_list)
    for k in range(N_CORES):
        fs = fpad[k * rows_core : (k + 1) * rows_core]
        lw = np.empty((P, T), dtype=np.int32)
        lw[:, :J16] = lab16[k * J16 * P : (k + 1) * J16 * P].reshape(J16, P).T
        if J8:
            lw[:, J16:] = lab8[k * J8 * P : (k + 1) * J8 * P].reshape(J8, P).T
        maps.append(
            {"features": fs, "labels": np.ascontiguousarray(lw), "centers": centers}
        )
    return maps, slots_list


def run(features, centers, labels, trace=False):
    maps, slots_list = _prepare(features, centers, labels)
    nc = _build(slots_list)
    res = run_bass_kernel_spmd(
        nc, maps, core_ids=list(range(N_CORES)), trace=trace
    )
    total = 0.0
    for r in res.results:
        total += float(np.asarray(r["out"]).astype(np.float64).sum())
    return np.float32(total / B), res


def kernel(features, centers, labels):
    last_err = None
    for _ in range(3):
        try:
            loss, _ = run(features, centers, labels)
            return loss
        except Exception as e:  # noqa: BLE001
            last_err = e
    raise last_err
